# revision 36
# baseline (speedup 1.0000x reference)
"""KalmanLSTM on 8 Trainium2 NeuronCores (Bass/Tile).

Key structural facts exploited:
  * During the 15 history steps the Kalman covariance P_t and gain K_t are
    batch-independent (P_0 is shared), so X_t is a shared linear map of the
    stacked history:  feat_t = tanh(Omega_t @ Zstack + cf_b).  No per-element
    Kalman work at all during history.
  * In the prediction phase, P00/P03/P33 decompose into host constants plus
    causal convolutions (lower-triangular 25x25 matmuls) of per-element
    command products, because P_s = F^s P_H F^s^T + sum F^(s-k) Q_k F^(s-k)^T
    and F is block-diagonal.  mu comes directly from the (linear) X recursion.
  * Only the LSTM is a true per-element nonlinear recurrence.  It runs in
    bf16: per-gate-type sigmoid/tanh activations straight from PSUM, and the
    cell/hidden updates as plain tensor_tensor ops split per batch-half and
    balanced across the Vector and GpSimd engines.

Layout: batch 32768 -> 8 cores x 4096 columns; per core 8 chunks of 512
columns mapped to (half H = j//4, lane L = j%4); 32-partition state tiles are
packed 4 lanes deep so every DVE/ACT instruction runs 128 partitions wide.
"""

import os
import sys

import numpy as np

sys.path.insert(0, "/opt/trn_rl_repo")

DT = 0.2
NV = 6
FEAT = 32
T = 16
NCORES = 8
BATCH = 32768
NPC = BATCH // NCORES          # columns per core
NHALF = NPC // 2               # 2048
CHUNK = 512


# ---------------------------------------------------------------------------
# host-side shared linear algebra (float64)
# ---------------------------------------------------------------------------

def _constants():
    H = np.zeros((2, NV)); H[0, 0] = 1; H[1, 3] = 1
    F = np.eye(NV)
    F[0, 1] = DT; F[0, 2] = DT * DT / 2; F[1, 2] = DT
    F[3, 4] = DT; F[3, 5] = DT * DT / 2; F[4, 5] = DT
    G = np.zeros((NV, 2))
    G[0, 0] = DT ** 3 / 6; G[1, 0] = DT * DT / 2; G[2, 0] = DT
    G[3, 1] = DT ** 3 / 6; G[4, 1] = DT * DT / 2; G[5, 1] = DT
    return H, F, G, G.copy()


def _host_precompute(inp, npred):
    H, F, G, Bm = _constants()
    tanhG = np.tanh(np.asarray(inp['coef_G'], np.float64))
    GR = np.asarray(inp['GR'], np.float64)
    R = GR[:, None] * GR[None, :]
    Gs_hist = G * tanhG * np.asarray(inp['jerk_std'], np.float64)
    Q_hist = Gs_hist @ Gs_hist.T
    GtG = G * tanhG

    diagv = np.array([
        float(np.asarray(inp['position_std_x'])[0]) ** 2,
        float(np.asarray(inp['velocity_std_x'])[0]) ** 2,
        float(np.asarray(inp['acceleration_std_x'])[0]) ** 2,
        float(np.asarray(inp['position_std_y'])[0]) ** 2,
        float(np.asarray(inp['velocity_std_y'])[0]) ** 2,
        float(np.asarray(inp['acceleration_std_y'])[0]) ** 2,
    ])
    P = np.diag(diagv)
    Id = np.eye(NV)

    # Phi_t: X_t = Phi_t @ Zstack   (Zstack rows 0..15 = x_t, 16..31 = y_t)
    Phi = np.zeros((NV, 2 * T))
    Phi[0, 0] = 1.0
    Phi[1, 1] = 1.0 / DT; Phi[1, 0] = -1.0 / DT
    Phi[3, 17] = 1.0 / DT; Phi[3, 16] = -1.0 / DT
    Phis = [Phi.copy()]
    for t in range(1, T):
        Pm = F @ P @ F.T + Q_hist
        S = H @ Pm @ H.T + R
        K = Pm @ H.T @ np.linalg.inv(S)
        ImKH = Id - K @ H
        P = ImKH @ Pm @ ImKH.T + K @ R @ K.T
        E = np.zeros((2, 2 * T)); E[0, t] = 1; E[1, 16 + t] = 1
        Phi = (ImKH @ F) @ Phi + K @ E
        Phis.append(Phi.copy())
    P15 = P
    Lam = Phis[T - 1]

    cfW = np.asarray(inp['cf_W'], np.float64)
    cfb = np.asarray(inp['cf_b'], np.float64)
    coW = np.asarray(inp['co_W'], np.float64)
    cob = np.asarray(inp['co_b'], np.float64)

    alpha = np.empty(npred); gamma = np.empty(npred)
    A00 = np.empty(npred); A03 = np.empty(npred); A33 = np.empty(npred)
    Fm = np.eye(NV)
    for m in range(npred):
        alpha[m] = Fm[0, :] @ GtG[:, 0]
        gamma[m] = Fm[3, :] @ GtG[:, 1]
        Fm = Fm @ F
    Fs = np.eye(NV)
    for s in range(npred):
        Fs = Fs @ F
        PP = Fs @ P15 @ Fs.T
        A00[s] = PP[0, 0]; A03[s] = PP[0, 3]; A33[s] = PP[3, 3]

    L1 = np.zeros((npred, npred)); L3 = np.zeros((npred, npred))
    L13 = np.zeros((npred, npred))
    for s in range(npred):
        for k in range(s + 1):
            L1[s, k] = alpha[s - k] ** 2
            L3[s, k] = gamma[s - k] ** 2
            L13[s, k] = alpha[s - k] * gamma[s - k]

    # bias-folding for co_b (X-update bias bx and cmd34 bias):
    bx = Bm @ cob[0:2]
    vb = np.zeros(NV)
    cfb_pred = np.zeros((npred, FEAT))
    mu_bias = np.zeros((npred, 2))
    for s in range(npred):
        cfb_pred[s] = cfb + cfW @ vb        # feat_s uses X_{s-1}
        vb = F @ vb + bx
        mu_bias[s, 0] = vb[0]; mu_bias[s, 1] = vb[3]

    f32 = np.float32
    import ml_dtypes
    bf16 = ml_dtypes.bfloat16

    # ---- device constant tensors ----
    omegasT = np.zeros((32, 32 * (T - 1)), f32)
    for t in range(1, T):
        Om = cfW @ Phis[t - 1]              # (32, 32)
        omegasT[:, 32 * (t - 1):32 * t] = Om.T
    omegasT = omegasT.astype(bf16)

    lamT4 = np.zeros((32, 512), f32)
    for L in range(4):
        for d in range(NV):
            lamT4[:, 128 * L + 32 * L + d] = Lam[d, :]

    cfwTbd = np.zeros((128, 128), f32)
    fTbd = np.zeros((128, 128), f32)
    for L in range(4):
        cfwTbd[32 * L:32 * L + NV, 32 * L:32 * L + 32] = cfW.T
        fTbd[32 * L:32 * L + NV, 32 * L:32 * L + NV] = F.T
    cfwTbd_bf = cfwTbd.astype(bf16)

    BcoT = (Bm @ coW[0:2, :]).T             # (32, 6)
    bcobd = np.zeros((128, 128), f32)
    for L in range(4):
        bcobd[32 * L:32 * L + 32, 32 * L:32 * L + NV] = BcoT
    # Auxiliary output rows packed into the X-update psum via extra lhsT
    # columns (all 32-aligned-base-free; these rows never feed back because
    # the corresponding K-rows of fTbd/cfwTbd are zero):
    #   cols  8+L : c3+c4 command sum   (for the (c3+c4)^2 trick)
    #   cols 16+L : c3 command
    #   cols 20+L : c4 command
    #   cols 24+L : mu_x = X[0] dup,  cols 28+L : mu_y = X[3] dup
    for L in range(4):
        fTbd[:, 24 + L] = fTbd[:, 32 * L + 0]
        fTbd[:, 28 + L] = fTbd[:, 32 * L + 3]
        bcobd[:, 24 + L] = bcobd[:, 32 * L + 0]
        bcobd[:, 28 + L] = bcobd[:, 32 * L + 3]
        bcobd[32 * L:32 * L + 32, 8 + L] = coW[2, :] + coW[3, :]
        bcobd[32 * L:32 * L + 32, 16 + L] = coW[2, :]
        bcobd[32 * L:32 * L + 32, 20 + L] = coW[3, :]
    bcobd = bcobd.astype(bf16)

    # gates: reference order i, f, g, o; sigmoid applied directly on-device
    Wih = np.asarray(inp['lstm_Wih'], np.float64)
    Whh = np.asarray(inp['lstm_Whh'], np.float64)
    bsum = (np.asarray(inp['lstm_bih'], np.float64)
            + np.asarray(inp['lstm_bhh'], np.float64))
    gscale = np.array([1.0, 1.0, 1.0, 1.0])
    wihbd = np.zeros((128, 512), f32)
    whhbd = np.zeros((128, 512), f32)
    for Tg in range(4):
        wb = gscale[Tg] * Wih[32 * Tg:32 * Tg + 32, :]
        hb = gscale[Tg] * Whh[32 * Tg:32 * Tg + 32, :]
        for L in range(4):
            wihbd[32 * L:32 * L + 32, 128 * Tg + 32 * L:128 * Tg + 32 * L + 32] = wb.T
            whhbd[32 * L:32 * L + 32, 128 * Tg + 32 * L:128 * Tg + 32 * L + 32] = hb.T
    wihbd = wihbd.astype(bf16)
    whhbd = whhbd.astype(bf16)

    # command history lives on 32-aligned partition blocks (engine ops need
    # 32-aligned base partitions): cmdA rows 0..np-1 = c3, 32..32+np-1 = c4,
    # 64.. = c3+c4.  conv contracts over 96 rows with zeros in the gaps.
    assert npred <= 32
    nconv = 3 * npred
    convT = np.zeros((96, nconv), f32)
    convT[0:npred, 0:npred] = L1.T
    convT[32:32 + npred, npred:2 * npred] = L3.T
    # P03 = sum L13 * c3*c4 with c3*c4 = ((c3+c4)^2 - c3^2 - c4^2)/2
    convT[0:npred, 2 * npred:] = -0.5 * L13.T
    convT[32:32 + npred, 2 * npred:] = -0.5 * L13.T
    convT[64:64 + npred, 2 * npred:] = 0.5 * L13.T

    nbias = 1 + npred + 4 + 3
    biases = np.zeros((128, nbias), f32)
    tile4 = lambda v: np.tile(np.asarray(v, np.float64), 4)
    biases[:, 0] = tile4(cfb)
    for s in range(npred):
        biases[:, 1 + s] = tile4(cfb_pred[s])
    for Tg in range(4):
        biases[:, 1 + npred + Tg] = tile4(gscale[Tg] * bsum[32 * Tg:32 * Tg + 32])
    cA = 1 + npred + 4
    biases[0:npred, cA] = A00
    biases[npred:2 * npred, cA] = A33
    biases[2 * npred:3 * npred, cA] = A03
    biases[0:npred, cA + 1] = cob[2]
    biases[32:32 + npred, cA + 1] = cob[3]
    biases[64:64 + npred, cA + 1] = cob[2] + cob[3]

    consts = dict(omegasT=omegasT, lamT4=lamT4, cfwTbd=cfwTbd_bf, fTbd=fTbd,
                  bcobd=bcobd, wihbd=wihbd, whhbd=whhbd,
                  biases=biases.astype(f32), convT=convT)
    return consts, mu_bias


# ---------------------------------------------------------------------------
# device program
# ---------------------------------------------------------------------------

_NC_CACHE = {}


def _patched_tile_context(tile, bass, mybir):
    """TileContext whose exit drain splits its semaphore waits across
    single-wait NOPs.  The stock `_drain_and_barrier` attaches every
    engine/queue semaphore wait to one Drain instruction; this walrus build
    rejects >2 sync waits per instruction (codegen NCC_INLA001 'Too many
    sync wait commands'), so pre-satisfy them on dedicated NOPs instead."""
    from concourse.vector_clock import ScopedClock

    class SplitDrainTileContext(tile.TileContext):
        _MAXW = 1

        def _commit_instruction(self, inst, lazy_reg_writes=True):
            # this walrus build rejects instructions carrying more than one
            # sync wait ("Too many sync wait commands"); spill the excess
            # onto same-engine NOPs committed immediately before.
            si = getattr(inst, "sync_info", None)
            if si is not None and len(si.on_wait) > self._MAXW:
                waits = list(si.on_wait)
                excess, keep = waits[:-self._MAXW], waits[-self._MAXW:]
                for w in excess:
                    nop = mybir.InstNoOp(
                        name=self.nc.get_next_instruction_name(),
                        ins=[], outs=[])
                    nop.engine = inst.engine
                    nop.sync_info = mybir.SyncInfo(on_wait=[w], on_update=[])
                    super()._commit_instruction(nop, lazy_reg_writes)
                inst.sync_info = mybir.SyncInfo(
                    on_wait=keep, on_update=list(si.on_update))
            return super()._commit_instruction(inst, lazy_reg_writes)

        def _drain_and_barrier(self, tick_clock, wait_clock):
            probe = self.nc.sync.nop()
            wait_clock.add_sem_waits(
                probe.ins, ScopedClock({None: tick_clock.global_clock}))
            si = probe.ins.sync_info
            waits = list(si.on_wait) if si is not None else []
            if len(waits) > 1:
                probe.ins.sync_info = mybir.SyncInfo(
                    on_wait=waits[:1], on_update=[])
                for i in range(1, len(waits)):
                    extra = self.nc.sync.nop()
                    extra.ins.sync_info = mybir.SyncInfo(
                        on_wait=waits[i:i + 1], on_update=[])
            self.nc.sync.drain()
            self.nc.all_engine_barrier()
            assert self.sems is not None
            popped = self.nc._tile_sem_poison_stack.pop()
            assert popped is self._sem_poison
            self.nc.clear_and_free_semaphores(
                list(self.sems.allocated().values()))
            self.nc.all_engine_barrier()

    return SplitDrainTileContext


def _build_nc(npred):
    from concourse import bass, mybir, tile

    f32 = mybir.dt.float32
    bf = mybir.dt.bfloat16
    f32r = mybir.dt.float32r
    Tanh = mybir.ActivationFunctionType.Tanh
    Sigmoid = mybir.ActivationFunctionType.Sigmoid
    Ident = mybir.ActivationFunctionType.Identity
    add = mybir.AluOpType.add
    mult = mybir.AluOpType.mult
    nconv = 3 * npred
    cA = 1 + npred + 4

    nc = bass.Bass(target_bir_lowering=False, debug=False)

    zstack_d = nc.declare_dram_parameter("zstack", [32, NPC], f32, False)
    zstackh_d = nc.declare_dram_parameter("zstackh", [32, NPC], bf, False)
    omegasT_d = nc.declare_dram_parameter("omegasT", [32, 32 * (T - 1)], bf,
                                          False)
    lamT4_d = nc.declare_dram_parameter("lamT4", [32, 512], f32, False)
    cfwTbd_d = nc.declare_dram_parameter("cfwTbd", [128, 128], bf, False)
    fTbd_d = nc.declare_dram_parameter("fTbd", [128, 128], f32, False)
    bcobd_d = nc.declare_dram_parameter("bcobd", [128, 128], bf, False)
    wihbd_d = nc.declare_dram_parameter("wihbd", [128, 512], bf, False)
    whhbd_d = nc.declare_dram_parameter("whhbd", [128, 512], bf, False)
    biases_d = nc.declare_dram_parameter("biases", [128, cA + 3], f32, False)
    convT_d = nc.declare_dram_parameter("convT", [96, nconv], f32, False)
    out_d = nc.declare_dram_parameter("out", [5 * npred, NPC], f32, True)

    TC = _patched_tile_context(tile, bass, mybir)
    with TC(nc) as tc:
        with (
            tc.tile_pool(name="const", bufs=1) as cp_,
            tc.tile_pool(name="state", bufs=1) as sp_,
            tc.tile_pool(name="fpsum", bufs=2, space="PSUM") as fpool,
            tc.tile_pool(name="gpsum", bufs=4, space="PSUM") as gpool,
            tc.tile_pool(name="xpsum", bufs=2, space="PSUM") as xpool,
            tc.tile_pool(name="dram", bufs=1, space="DRAM") as dpool,
        ):
            def load_const(dram, shape, dtype):
                t_ = cp_.tile(shape, dtype, tag=dram.name)
                nc.sync.dma_start(t_[:], dram[:])
                return t_

            zsb = load_const(zstack_d, [32, NPC], f32)
            zsbh = load_const(zstackh_d, [32, NPC], bf)
            omg = load_const(omegasT_d, [32, 32 * (T - 1)], bf)
            lamT4 = load_const(lamT4_d, [32, 512], f32)
            cfwTbd = load_const(cfwTbd_d, [128, 128], bf)
            fTbd = load_const(fTbd_d, [128, 128], f32)
            bcobd = load_const(bcobd_d, [128, 128], bf)
            wihbd = load_const(wihbd_d, [128, 512], bf)
            whhbd = load_const(whhbd_d, [128, 512], bf)
            biases = load_const(biases_d, [128, cA + 3], f32)
            convT = load_const(convT_d, [96, nconv], f32)

            feat4 = sp_.tile([128, 2 * CHUNK], bf, tag="feat4")
            h4 = sp_.tile([128, 2 * CHUNK], bf, tag="h4")
            c4 = sp_.tile([128, 2 * CHUNK], bf, tag="c4")
            tc4 = sp_.tile([128, 2 * CHUNK], bf, tag="tc4")
            t1 = sp_.tile([128, 2 * CHUNK], bf, tag="t1")
            gact = [sp_.tile([128, 2 * CHUNK], bf, tag=f"gact{i}",
                             name=f"gact{i}") for i in range(4)]
            xsb = sp_.tile([128, 2 * CHUNK], f32, tag="xsb")
            xsbh = sp_.tile([128, 2 * CHUNK], bf, tag="xsbh")
            cmdA = sp_.tile([96, NPC], f32, tag="cmdA")
            qS = sp_.tile([96, NPC], f32, tag="qS")
            musb = sp_.tile([2 * npred, NPC], f32, tag="musb")
            poutsb = sp_.tile([nconv, NPC], f32, tag="poutsb")
            # per-step staging of [csum(4); c3(4); c4(4); mu_x(4); mu_y(4)]
            # rows (xsb partitions 8..31) -> one DRAM row per step, gathered
            # into row-major layouts by 5 large DMAs at the end
            stage = dpool.tile([npred, 24, 2 * CHUNK], f32, tag="stage")

            nc.vector.memset(h4[:], 0.0)
            nc.vector.memset(c4[:], 0.0)
            nc.vector.memset(cmdA[:], 0.0)

            def lstm_step(bias_col, split_gact=False):
                # one psum tile + activation per (gate-type, half): 1-bank
                # tiles give 4-deep buffering so matmul/activation pairs of
                # adjacent steps stay in flight, and half 0's cell update
                # starts before half 1's matmuls finish
                for Tg in (0, 2, 1, 3):
                    func = Tanh if Tg == 2 else Sigmoid
                    bias = biases[:, 1 + npred + Tg:2 + npred + Tg]
                    for H in range(2):
                        sl = slice(CHUNK * H, CHUNK * (H + 1))
                        gp = gpool.tile([128, CHUNK], f32, tag="gp")
                        nc.tensor.matmul(gp[:],
                                         wihbd[:, 128 * Tg:128 * (Tg + 1)],
                                         feat4[:, sl], start=True, stop=False)
                        nc.tensor.matmul(gp[:],
                                         whhbd[:, 128 * Tg:128 * (Tg + 1)],
                                         h4[:, sl], start=False, stop=True)
                        nc.scalar.activation(gact[Tg][:, sl], gp[:],
                                             func, bias=bias)
                # c' = sig(f)*c + sig(i)*tanh(g) ;  h = sig(o)*tanh(c')
                # processed per batch-half so half 0's h is ready (and the
                # next step's matmuls can start) while half 1 still updates;
                # t1 and u run concurrently on DVE/GpSimd
                engs = [(nc.vector, nc.gpsimd, nc.vector),
                        (nc.gpsimd, nc.vector, nc.gpsimd)]
                for H in range(2):
                    sh = slice(CHUNK * H, CHUNK * (H + 1))
                    e_t1, e_u, e_c = engs[H]
                    e_t1.tensor_mul(t1[:, sh], gact[0][:, sh], gact[2][:, sh])
                    e_u.tensor_mul(c4[:, sh], gact[1][:, sh], c4[:, sh])
                    e_c.tensor_add(c4[:, sh], c4[:, sh], t1[:, sh])
                    nc.scalar.activation(tc4[:, sh], c4[:, sh], Tanh)
                    nc.gpsimd.tensor_mul(h4[:, sh], gact[3][:, sh],
                                         tc4[:, sh])

            # ---- history ----
            for t in range(1, T):
                for H in range(2):
                    sh = slice(CHUNK * H, CHUNK * (H + 1))
                    fp = fpool.tile([128, CHUNK], f32, tag="fp")
                    for L in range(4):
                        j = 4 * H + L
                        nc.tensor.matmul(
                            fp[32 * L:32 * (L + 1), :],
                            omg[:, 32 * (t - 1):32 * t],
                            zsbh[:, CHUNK * j:CHUNK * (j + 1)],
                            start=True, stop=True, tile_position=(0, 32 * L))
                    nc.scalar.activation(feat4[:, sh], fp[:], Tanh,
                                         bias=biases[:, 0:1])
                lstm_step(0, split_gact=True)

            # ---- X_15 = Lam @ Zstack (lane-packed, zero-filled rows) ----
            for H in range(2):
                xp = xpool.tile([128, CHUNK], f32, tag="xp")
                for L in range(4):
                    j = 4 * H + L
                    nc.tensor.matmul(
                        xp[:], lamT4[:, 128 * L:128 * (L + 1)],
                        zsb[:, CHUNK * j:CHUNK * (j + 1)],
                        start=(L == 0), stop=(L == 3))
                sl = slice(CHUNK * H, CHUNK * (H + 1))
                nc.vector.tensor_copy(xsb[:, sl], xp[:])
                nc.vector.tensor_copy(xsbh[:, sl], xp[:])

            # ---- prediction ----
            for s in range(1, npred + 1):
                for H in range(2):
                    sl = slice(CHUNK * H, CHUNK * (H + 1))
                    fp = fpool.tile([128, CHUNK], f32, tag="fp")
                    nc.tensor.matmul(fp[:], cfwTbd[:],
                                     xsbh[:, sl], start=True, stop=True)
                    nc.scalar.activation(feat4[:, sl], fp[:], Tanh,
                                         bias=biases[:, s:s + 1])
                lstm_step(s, split_gact=True)

                xps = []
                for H in range(2):
                    sl = slice(CHUNK * H, CHUNK * (H + 1))
                    # X update + command/mu rows in one psum group
                    xp = xpool.tile([128, CHUNK], f32, tag="xp")
                    nc.tensor.matmul(xp[:], fTbd[:], xsb[:, sl],
                                     start=True, stop=False)
                    nc.tensor.matmul(xp[:], bcobd[:], h4[:, sl],
                                     start=False, stop=True)
                    # the bf16 shadow feeds the next step's feat matmul —
                    # issue it before either off-chain fp32 copy queues
                    nc.vector.tensor_copy(xsbh[:, sl], xp[:])
                    xps.append(xp)
                for H in range(2):
                    sl = slice(CHUNK * H, CHUNK * (H + 1))
                    nc.vector.tensor_copy(xsb[:, sl], xps[H][:])
                nc.sync.dma_start(stage[s - 1], xsb[8:32, :])

            # ---- epilogue: gather staged rows, products, conv ----
            def gather(dst_ap, r0):
                # DMA APs balance to at most 3 dims -> one DMA per half;
                # alternate the two HWDGE engines so the gathers overlap
                for H in range(2):
                    src_ap = stage[:, r0:r0 + 4, CHUNK * H:CHUNK * (H + 1)]
                    eng = nc.sync if H == 0 else nc.scalar
                    eng.dma_start(
                        dst_ap[:, 2048 * H:2048 * (H + 1)], src_ap)

            gather(cmdA[64:64 + npred, :], 0)    # c3+c4
            gather(cmdA[0:npred, :], 8)          # c3
            gather(cmdA[32:32 + npred, :], 12)   # c4
            gather(musb[0:npred, :], 16)         # mu_x
            gather(musb[npred:2 * npred, :], 20)  # mu_y
            nc.vector.tensor_scalar_add(cmdA[:], cmdA[:],
                                        biases[0:96, cA + 1:cA + 2])
            nc.vector.tensor_mul(qS[:], cmdA[:], cmdA[:])
            for j in range(8):
                sl = slice(CHUNK * j, CHUNK * (j + 1))
                pp = gpool.tile([nconv, CHUNK], f32, tag="gp")
                nc.tensor.matmul(pp[0:nconv, :], convT[:], qS[:, sl],
                                 start=True, stop=True)
                nc.scalar.activation(poutsb[:, sl], pp[0:nconv, :], Ident,
                                     bias=biases[0:nconv, cA:cA + 1])
            nc.scalar.dma_start(out_d[0:2 * npred, :], musb[:])
            nc.sync.dma_start(out_d[2 * npred:, :], poutsb[:])

    return nc


def _get_nc(npred):
    if npred not in _NC_CACHE:
        _NC_CACHE[npred] = _build_nc(npred)
    return _NC_CACHE[npred]


_RUNNER_CACHE = {}


def _get_runner(npred):
    """Cached jitted SPMD executor.  Mirrors the multi-core path of
    concourse.bass2jax.run_bass_via_pjrt, but builds the shard_map-jitted
    callable once so repeat kernel() calls skip retracing (~1.3 s/call)."""
    if npred in _RUNNER_CACHE:
        return _RUNNER_CACHE[npred]

    import jax
    from jax.sharding import Mesh, PartitionSpec
    from jax.experimental.shard_map import shard_map
    from concourse import bass2jax, mybir

    bass2jax.install_neuronx_cc_hook()
    nc = _get_nc(npred)

    partition_name = (nc.partition_id_tensor.name
                      if nc.partition_id_tensor else None)
    in_names, out_names, out_avals, zero_shapes = [], [], [], []
    for alloc in nc.m.functions[0].allocations:
        if not isinstance(alloc, mybir.MemoryLocationSet):
            continue
        name = alloc.memorylocations[0].name
        if alloc.kind == "ExternalInput":
            if name != partition_name:
                in_names.append(name)
        elif alloc.kind == "ExternalOutput":
            shape = tuple(alloc.tensor_shape)
            dtype = mybir.dt.np(alloc.dtype)
            out_names.append(name)
            out_avals.append(jax.core.ShapedArray(shape, dtype))
            zero_shapes.append((shape, dtype))
    n_params = len(in_names)
    n_outs = len(out_names)
    all_in_names = list(in_names) + list(out_names)
    if partition_name is not None:
        all_in_names.append(partition_name)
    donate = tuple(range(n_params, n_params + n_outs))

    def _body(*args):
        operands = list(args)
        if partition_name is not None:
            operands.append(bass2jax.partition_id_tensor())
        outs = bass2jax._bass_exec_p.bind(
            *operands,
            out_avals=tuple(out_avals),
            in_names=tuple(all_in_names),
            out_names=tuple(out_names),
            lowering_input_output_aliases=(),
            sim_require_finite=True,
            sim_require_nnan=True,
            nc=nc,
        )
        return tuple(outs)

    devices = jax.devices()[:NCORES]
    mesh = Mesh(np.asarray(devices), ("core",))
    in_specs = (PartitionSpec("core"),) * (n_params + n_outs)
    out_specs = (PartitionSpec("core"),) * n_outs
    sharded = jax.jit(
        shard_map(_body, mesh=mesh, in_specs=in_specs, out_specs=out_specs,
                  check_rep=False),
        donate_argnums=donate, keep_unused=True)

    def run(in_maps):
        per_core = [[np.asarray(m[name]) for name in in_names]
                    for m in in_maps]
        concat_in = [
            np.concatenate([per_core[c][i] for c in range(NCORES)], axis=0)
            for i in range(n_params)
        ]
        concat_zeros = [np.zeros((NCORES * s[0], *s[1:]), d)
                        for s, d in zero_shapes]
        out_arrs = sharded(*concat_in, *concat_zeros)
        return [
            {name: np.asarray(out_arrs[i]).reshape(
                NCORES, *out_avals[i].shape)[c]
             for i, name in enumerate(out_names)}
            for c in range(NCORES)
        ]

    _RUNNER_CACHE[npred] = run
    return run


# ---------------------------------------------------------------------------
# entry point
# ---------------------------------------------------------------------------

LAST_RESULTS = None


def kernel(
    hist, position_std_x, position_std_y, velocity_std_x, velocity_std_y,
    acceleration_std_x, acceleration_std_y, jerk_std, coef_G, GR,
    cf_W, cf_b, lstm_Wih, lstm_Whh, lstm_bih, lstm_bhh, co_W, co_b, len_pred,
):
    global LAST_RESULTS
    import ml_dtypes

    npred = int(np.asarray(len_pred))
    inp = dict(hist=hist, position_std_x=position_std_x,
               position_std_y=position_std_y, velocity_std_x=velocity_std_x,
               velocity_std_y=velocity_std_y,
               acceleration_std_x=acceleration_std_x,
               acceleration_std_y=acceleration_std_y, jerk_std=jerk_std,
               coef_G=coef_G, GR=GR, cf_W=cf_W, cf_b=cf_b,
               lstm_Wih=lstm_Wih, lstm_Whh=lstm_Whh, lstm_bih=lstm_bih,
               lstm_bhh=lstm_bhh, co_W=co_W, co_b=co_b)

    hist = np.asarray(hist, np.float32)
    Tl, B, _ = hist.shape
    assert (Tl, B) == (T, BATCH), (Tl, B)

    consts, mu_bias = _host_precompute(inp, npred)
    zs = np.concatenate([hist[:, :, 0], hist[:, :, 1]], axis=0)  # (32, B)
    zs = np.ascontiguousarray(zs, np.float32)

    in_maps = []
    for cidx in range(NCORES):
        m = dict(consts)
        zslice = np.ascontiguousarray(zs[:, NPC * cidx:NPC * (cidx + 1)])
        m["zstack"] = zslice
        m["zstackh"] = zslice.astype(ml_dtypes.bfloat16)
        in_maps.append(m)

    run = _get_runner(npred)
    results = run(in_maps)
    LAST_RESULTS = results

    mu_x = np.empty((npred, BATCH), np.float64)
    mu_y = np.empty((npred, BATCH), np.float64)
    p00 = np.empty((npred, BATCH), np.float64)
    p33 = np.empty((npred, BATCH), np.float64)
    p03 = np.empty((npred, BATCH), np.float64)
    for cidx in range(NCORES):
        o = np.asarray(results[cidx]["out"], np.float64)
        sl = slice(NPC * cidx, NPC * (cidx + 1))
        mu_x[:, sl] = o[0:npred]
        mu_y[:, sl] = o[npred:2 * npred]
        p00[:, sl] = o[2 * npred:3 * npred]
        p33[:, sl] = o[3 * npred:4 * npred]
        p03[:, sl] = o[4 * npred:5 * npred]

    mu_x += mu_bias[:, 0:1]
    mu_y += mu_bias[:, 1:2]
    sx = np.sqrt(np.maximum(p00, 0.0))
    sy = np.sqrt(np.maximum(p33, 0.0))
    rho = p03 / np.maximum(sx * sy, 1e-300)
    out = np.stack([mu_x, mu_y, sx, sy, rho], axis=2)
    return out.astype(np.float32)


# revision 39
# speedup vs baseline: 1.0032x; 1.0032x over previous
"""KalmanLSTM on 8 Trainium2 NeuronCores (Bass/Tile).

Key structural facts exploited:
  * During the 15 history steps the Kalman covariance P_t and gain K_t are
    batch-independent (P_0 is shared), so X_t is a shared linear map of the
    stacked history:  feat_t = tanh(Omega_t @ Zstack + cf_b).  No per-element
    Kalman work at all during history.
  * In the prediction phase, P00/P03/P33 decompose into host constants plus
    causal convolutions (lower-triangular 25x25 matmuls) of per-element
    command products, because P_s = F^s P_H F^s^T + sum F^(s-k) Q_k F^(s-k)^T
    and F is block-diagonal.  mu comes directly from the (linear) X recursion.
  * Only the LSTM is a true per-element nonlinear recurrence.  It runs in
    bf16: per-gate-type sigmoid/tanh activations straight from PSUM, and the
    cell/hidden updates as plain tensor_tensor ops split per batch-half and
    balanced across the Vector and GpSimd engines.

Layout: batch 32768 -> 8 cores x 4096 columns; per core 8 chunks of 512
columns mapped to (half H = j//4, lane L = j%4); 32-partition state tiles are
packed 4 lanes deep so every DVE/ACT instruction runs 128 partitions wide.
"""

import os
import sys

import numpy as np

sys.path.insert(0, "/opt/trn_rl_repo")

DT = 0.2
NV = 6
FEAT = 32
T = 16
NCORES = 8
BATCH = 32768
NPC = BATCH // NCORES          # columns per core
NHALF = NPC // 2               # 2048
CHUNK = 512


# ---------------------------------------------------------------------------
# host-side shared linear algebra (float64)
# ---------------------------------------------------------------------------

def _constants():
    H = np.zeros((2, NV)); H[0, 0] = 1; H[1, 3] = 1
    F = np.eye(NV)
    F[0, 1] = DT; F[0, 2] = DT * DT / 2; F[1, 2] = DT
    F[3, 4] = DT; F[3, 5] = DT * DT / 2; F[4, 5] = DT
    G = np.zeros((NV, 2))
    G[0, 0] = DT ** 3 / 6; G[1, 0] = DT * DT / 2; G[2, 0] = DT
    G[3, 1] = DT ** 3 / 6; G[4, 1] = DT * DT / 2; G[5, 1] = DT
    return H, F, G, G.copy()


def _host_precompute(inp, npred):
    H, F, G, Bm = _constants()
    tanhG = np.tanh(np.asarray(inp['coef_G'], np.float64))
    GR = np.asarray(inp['GR'], np.float64)
    R = GR[:, None] * GR[None, :]
    Gs_hist = G * tanhG * np.asarray(inp['jerk_std'], np.float64)
    Q_hist = Gs_hist @ Gs_hist.T
    GtG = G * tanhG

    diagv = np.array([
        float(np.asarray(inp['position_std_x'])[0]) ** 2,
        float(np.asarray(inp['velocity_std_x'])[0]) ** 2,
        float(np.asarray(inp['acceleration_std_x'])[0]) ** 2,
        float(np.asarray(inp['position_std_y'])[0]) ** 2,
        float(np.asarray(inp['velocity_std_y'])[0]) ** 2,
        float(np.asarray(inp['acceleration_std_y'])[0]) ** 2,
    ])
    P = np.diag(diagv)
    Id = np.eye(NV)

    # Phi_t: X_t = Phi_t @ Zstack   (Zstack rows 0..15 = x_t, 16..31 = y_t)
    Phi = np.zeros((NV, 2 * T))
    Phi[0, 0] = 1.0
    Phi[1, 1] = 1.0 / DT; Phi[1, 0] = -1.0 / DT
    Phi[3, 17] = 1.0 / DT; Phi[3, 16] = -1.0 / DT
    Phis = [Phi.copy()]
    for t in range(1, T):
        Pm = F @ P @ F.T + Q_hist
        S = H @ Pm @ H.T + R
        K = Pm @ H.T @ np.linalg.inv(S)
        ImKH = Id - K @ H
        P = ImKH @ Pm @ ImKH.T + K @ R @ K.T
        E = np.zeros((2, 2 * T)); E[0, t] = 1; E[1, 16 + t] = 1
        Phi = (ImKH @ F) @ Phi + K @ E
        Phis.append(Phi.copy())
    P15 = P
    Lam = Phis[T - 1]

    cfW = np.asarray(inp['cf_W'], np.float64)
    cfb = np.asarray(inp['cf_b'], np.float64)
    coW = np.asarray(inp['co_W'], np.float64)
    cob = np.asarray(inp['co_b'], np.float64)

    alpha = np.empty(npred); gamma = np.empty(npred)
    A00 = np.empty(npred); A03 = np.empty(npred); A33 = np.empty(npred)
    Fm = np.eye(NV)
    for m in range(npred):
        alpha[m] = Fm[0, :] @ GtG[:, 0]
        gamma[m] = Fm[3, :] @ GtG[:, 1]
        Fm = Fm @ F
    Fs = np.eye(NV)
    for s in range(npred):
        Fs = Fs @ F
        PP = Fs @ P15 @ Fs.T
        A00[s] = PP[0, 0]; A03[s] = PP[0, 3]; A33[s] = PP[3, 3]

    L1 = np.zeros((npred, npred)); L3 = np.zeros((npred, npred))
    L13 = np.zeros((npred, npred))
    for s in range(npred):
        for k in range(s + 1):
            L1[s, k] = alpha[s - k] ** 2
            L3[s, k] = gamma[s - k] ** 2
            L13[s, k] = alpha[s - k] * gamma[s - k]

    # bias-folding for co_b (X-update bias bx and cmd34 bias):
    bx = Bm @ cob[0:2]
    vb = np.zeros(NV)
    cfb_pred = np.zeros((npred, FEAT))
    mu_bias = np.zeros((npred, 2))
    for s in range(npred):
        cfb_pred[s] = cfb + cfW @ vb        # feat_s uses X_{s-1}
        vb = F @ vb + bx
        mu_bias[s, 0] = vb[0]; mu_bias[s, 1] = vb[3]

    f32 = np.float32
    import ml_dtypes
    bf16 = ml_dtypes.bfloat16

    # ---- device constant tensors ----
    omegasT = np.zeros((32, 32 * (T - 1)), f32)
    for t in range(1, T):
        Om = cfW @ Phis[t - 1]              # (32, 32)
        omegasT[:, 32 * (t - 1):32 * t] = Om.T
    omegasT = omegasT.astype(bf16)

    lamT4 = np.zeros((32, 512), f32)
    for L in range(4):
        for d in range(NV):
            lamT4[:, 128 * L + 32 * L + d] = Lam[d, :]

    cfwTbd = np.zeros((128, 128), f32)
    fTbd = np.zeros((128, 128), f32)
    for L in range(4):
        cfwTbd[32 * L:32 * L + NV, 32 * L:32 * L + 32] = cfW.T
        fTbd[32 * L:32 * L + NV, 32 * L:32 * L + NV] = F.T
    cfwTbd_bf = cfwTbd.astype(bf16)

    BcoT = (Bm @ coW[0:2, :]).T             # (32, 6)
    bcobd = np.zeros((128, 128), f32)
    for L in range(4):
        bcobd[32 * L:32 * L + 32, 32 * L:32 * L + NV] = BcoT
    # Auxiliary output rows packed into the X-update psum via extra lhsT
    # columns (all 32-aligned-base-free; these rows never feed back because
    # the corresponding K-rows of fTbd/cfwTbd are zero):
    #   cols  8+L : c3+c4 command sum   (for the (c3+c4)^2 trick)
    #   cols 16+L : c3 command
    #   cols 20+L : c4 command
    #   cols 24+L : mu_x = X[0] dup,  cols 28+L : mu_y = X[3] dup
    for L in range(4):
        fTbd[:, 24 + L] = fTbd[:, 32 * L + 0]
        fTbd[:, 28 + L] = fTbd[:, 32 * L + 3]
        bcobd[:, 24 + L] = bcobd[:, 32 * L + 0]
        bcobd[:, 28 + L] = bcobd[:, 32 * L + 3]
        bcobd[32 * L:32 * L + 32, 8 + L] = coW[2, :] + coW[3, :]
        bcobd[32 * L:32 * L + 32, 16 + L] = coW[2, :]
        bcobd[32 * L:32 * L + 32, 20 + L] = coW[3, :]
    bcobd = bcobd.astype(bf16)

    # gates: reference order i, f, g, o; sigmoid applied directly on-device
    Wih = np.asarray(inp['lstm_Wih'], np.float64)
    Whh = np.asarray(inp['lstm_Whh'], np.float64)
    bsum = (np.asarray(inp['lstm_bih'], np.float64)
            + np.asarray(inp['lstm_bhh'], np.float64))
    gscale = np.array([1.0, 1.0, 1.0, 1.0])
    wihbd = np.zeros((128, 512), f32)
    whhbd = np.zeros((128, 512), f32)
    for Tg in range(4):
        wb = gscale[Tg] * Wih[32 * Tg:32 * Tg + 32, :]
        hb = gscale[Tg] * Whh[32 * Tg:32 * Tg + 32, :]
        for L in range(4):
            wihbd[32 * L:32 * L + 32, 128 * Tg + 32 * L:128 * Tg + 32 * L + 32] = wb.T
            whhbd[32 * L:32 * L + 32, 128 * Tg + 32 * L:128 * Tg + 32 * L + 32] = hb.T
    wihbd = wihbd.astype(bf16)
    whhbd = whhbd.astype(bf16)

    # command history lives on 32-aligned partition blocks (engine ops need
    # 32-aligned base partitions): cmdA rows 0..np-1 = c3, 32..32+np-1 = c4,
    # 64.. = c3+c4.  conv contracts over 96 rows with zeros in the gaps.
    assert npred <= 32
    nconv = 3 * npred
    convT = np.zeros((96, nconv), f32)
    convT[0:npred, 0:npred] = L1.T
    convT[32:32 + npred, npred:2 * npred] = L3.T
    # P03 = sum L13 * c3*c4 with c3*c4 = ((c3+c4)^2 - c3^2 - c4^2)/2
    convT[0:npred, 2 * npred:] = -0.5 * L13.T
    convT[32:32 + npred, 2 * npred:] = -0.5 * L13.T
    convT[64:64 + npred, 2 * npred:] = 0.5 * L13.T

    nbias = 1 + npred + 4 + 3
    biases = np.zeros((128, nbias), f32)
    tile4 = lambda v: np.tile(np.asarray(v, np.float64), 4)
    biases[:, 0] = tile4(cfb)
    for s in range(npred):
        biases[:, 1 + s] = tile4(cfb_pred[s])
    for Tg in range(4):
        biases[:, 1 + npred + Tg] = tile4(gscale[Tg] * bsum[32 * Tg:32 * Tg + 32])
    cA = 1 + npred + 4
    biases[0:npred, cA] = A00
    biases[npred:2 * npred, cA] = A33
    biases[2 * npred:3 * npred, cA] = A03
    biases[0:npred, cA + 1] = cob[2]
    biases[32:32 + npred, cA + 1] = cob[3]
    biases[64:64 + npred, cA + 1] = cob[2] + cob[3]

    consts = dict(omegasT=omegasT, lamT4=lamT4, cfwTbd=cfwTbd_bf, fTbd=fTbd,
                  bcobd=bcobd, wihbd=wihbd, whhbd=whhbd,
                  biases=biases.astype(f32), convT=convT)
    return consts, mu_bias


# ---------------------------------------------------------------------------
# device program
# ---------------------------------------------------------------------------

_NC_CACHE = {}


def _patched_tile_context(tile, bass, mybir):
    """TileContext whose exit drain splits its semaphore waits across
    single-wait NOPs.  The stock `_drain_and_barrier` attaches every
    engine/queue semaphore wait to one Drain instruction; this walrus build
    rejects >2 sync waits per instruction (codegen NCC_INLA001 'Too many
    sync wait commands'), so pre-satisfy them on dedicated NOPs instead."""
    from concourse.vector_clock import ScopedClock

    class SplitDrainTileContext(tile.TileContext):
        _MAXW = 1

        def _commit_instruction(self, inst, lazy_reg_writes=True):
            # this walrus build rejects instructions carrying more than one
            # sync wait ("Too many sync wait commands"); spill the excess
            # onto same-engine NOPs committed immediately before.
            si = getattr(inst, "sync_info", None)
            if si is not None and len(si.on_wait) > self._MAXW:
                waits = list(si.on_wait)
                excess, keep = waits[:-self._MAXW], waits[-self._MAXW:]
                for w in excess:
                    nop = mybir.InstNoOp(
                        name=self.nc.get_next_instruction_name(),
                        ins=[], outs=[])
                    nop.engine = inst.engine
                    nop.sync_info = mybir.SyncInfo(on_wait=[w], on_update=[])
                    super()._commit_instruction(nop, lazy_reg_writes)
                inst.sync_info = mybir.SyncInfo(
                    on_wait=keep, on_update=list(si.on_update))
            return super()._commit_instruction(inst, lazy_reg_writes)

        def _drain_and_barrier(self, tick_clock, wait_clock):
            probe = self.nc.sync.nop()
            wait_clock.add_sem_waits(
                probe.ins, ScopedClock({None: tick_clock.global_clock}))
            si = probe.ins.sync_info
            waits = list(si.on_wait) if si is not None else []
            if len(waits) > 1:
                probe.ins.sync_info = mybir.SyncInfo(
                    on_wait=waits[:1], on_update=[])
                for i in range(1, len(waits)):
                    extra = self.nc.sync.nop()
                    extra.ins.sync_info = mybir.SyncInfo(
                        on_wait=waits[i:i + 1], on_update=[])
            self.nc.sync.drain()
            self.nc.all_engine_barrier()
            assert self.sems is not None
            popped = self.nc._tile_sem_poison_stack.pop()
            assert popped is self._sem_poison
            self.nc.clear_and_free_semaphores(
                list(self.sems.allocated().values()))
            self.nc.all_engine_barrier()

    return SplitDrainTileContext


def _build_nc(npred):
    from concourse import bass, mybir, tile

    f32 = mybir.dt.float32
    bf = mybir.dt.bfloat16
    f32r = mybir.dt.float32r
    Tanh = mybir.ActivationFunctionType.Tanh
    Sigmoid = mybir.ActivationFunctionType.Sigmoid
    Ident = mybir.ActivationFunctionType.Identity
    add = mybir.AluOpType.add
    mult = mybir.AluOpType.mult
    nconv = 3 * npred
    cA = 1 + npred + 4

    nc = bass.Bass(target_bir_lowering=False, debug=False)

    zstack_d = nc.declare_dram_parameter("zstack", [32, NPC], f32, False)
    zstackh_d = nc.declare_dram_parameter("zstackh", [32, NPC], bf, False)
    omegasT_d = nc.declare_dram_parameter("omegasT", [32, 32 * (T - 1)], bf,
                                          False)
    lamT4_d = nc.declare_dram_parameter("lamT4", [32, 512], f32, False)
    cfwTbd_d = nc.declare_dram_parameter("cfwTbd", [128, 128], bf, False)
    fTbd_d = nc.declare_dram_parameter("fTbd", [128, 128], f32, False)
    bcobd_d = nc.declare_dram_parameter("bcobd", [128, 128], bf, False)
    wihbd_d = nc.declare_dram_parameter("wihbd", [128, 512], bf, False)
    whhbd_d = nc.declare_dram_parameter("whhbd", [128, 512], bf, False)
    biases_d = nc.declare_dram_parameter("biases", [128, cA + 3], f32, False)
    convT_d = nc.declare_dram_parameter("convT", [96, nconv], f32, False)
    out_d = nc.declare_dram_parameter("out", [5 * npred, NPC], f32, True)

    TC = _patched_tile_context(tile, bass, mybir)
    with TC(nc) as tc:
        with (
            tc.tile_pool(name="const", bufs=1) as cp_,
            tc.tile_pool(name="state", bufs=1) as sp_,
            tc.tile_pool(name="fpsum", bufs=2, space="PSUM") as fpool,
            tc.tile_pool(name="gpsum", bufs=4, space="PSUM") as gpool,
            tc.tile_pool(name="xpsum", bufs=2, space="PSUM") as xpool,
            tc.tile_pool(name="dram", bufs=1, space="DRAM") as dpool,
        ):
            def load_const(dram, shape, dtype):
                t_ = cp_.tile(shape, dtype, tag=dram.name)
                nc.sync.dma_start(t_[:], dram[:])
                return t_

            zsb = load_const(zstack_d, [32, NPC], f32)
            zsbh = load_const(zstackh_d, [32, NPC], bf)
            omg = load_const(omegasT_d, [32, 32 * (T - 1)], bf)
            lamT4 = load_const(lamT4_d, [32, 512], f32)
            cfwTbd = load_const(cfwTbd_d, [128, 128], bf)
            fTbd = load_const(fTbd_d, [128, 128], f32)
            bcobd = load_const(bcobd_d, [128, 128], bf)
            wihbd = load_const(wihbd_d, [128, 512], bf)
            whhbd = load_const(whhbd_d, [128, 512], bf)
            biases = load_const(biases_d, [128, cA + 3], f32)
            convT = load_const(convT_d, [96, nconv], f32)

            h4 = sp_.tile([128, 2 * CHUNK], bf, tag="h4")
            c4 = sp_.tile([128, 2 * CHUNK], bf, tag="c4")
            tc4 = sp_.tile([128, 2 * CHUNK], bf, tag="tc4")
            t1 = sp_.tile([128, 2 * CHUNK], bf, tag="t1")
            gact = [sp_.tile([128, 2 * CHUNK], bf, tag=f"gact{i}",
                             name=f"gact{i}") for i in range(4)]
            xsb = sp_.tile([128, 2 * CHUNK], f32, tag="xsb")
            xsbh = sp_.tile([128, 2 * CHUNK], bf, tag="xsbh")
            cmdA = sp_.tile([96, NPC], f32, tag="cmdA")
            qS = sp_.tile([96, NPC], f32, tag="qS")
            musb = sp_.tile([2 * npred, NPC], f32, tag="musb")
            poutsb = sp_.tile([nconv, NPC], f32, tag="poutsb")
            # per-step staging of [csum(4); c3(4); c4(4); mu_x(4); mu_y(4)]
            # rows (xsb partitions 8..31) -> one DRAM row per step, gathered
            # into row-major layouts by 5 large DMAs at the end
            stage = dpool.tile([npred, 24, 2 * CHUNK], f32, tag="stage")

            nc.vector.memset(h4[:], 0.0)
            nc.vector.memset(c4[:], 0.0)
            nc.vector.memset(cmdA[:], 0.0)

            def lstm_step(feat4, bias_col, split_gact=False):
                # one psum tile + activation per (gate-type, half): 1-bank
                # tiles give 4-deep buffering so matmul/activation pairs of
                # adjacent steps stay in flight, and half 0's cell update
                # starts before half 1's matmuls finish
                for Tg in (0, 2, 1, 3):
                    func = Tanh if Tg == 2 else Sigmoid
                    bias = biases[:, 1 + npred + Tg:2 + npred + Tg]
                    for H in range(2):
                        sl = slice(CHUNK * H, CHUNK * (H + 1))
                        gp = gpool.tile([128, CHUNK], f32, tag="gp")
                        nc.tensor.matmul(gp[:],
                                         wihbd[:, 128 * Tg:128 * (Tg + 1)],
                                         feat4[:, sl], start=True, stop=False)
                        nc.tensor.matmul(gp[:],
                                         whhbd[:, 128 * Tg:128 * (Tg + 1)],
                                         h4[:, sl], start=False, stop=True)
                        nc.scalar.activation(gact[Tg][:, sl], gp[:],
                                             func, bias=bias)
                # c' = sig(f)*c + sig(i)*tanh(g) ;  h = sig(o)*tanh(c')
                # processed per batch-half so half 0's h is ready (and the
                # next step's matmuls can start) while half 1 still updates;
                # t1 and u run concurrently on DVE/GpSimd
                engs = [(nc.vector, nc.gpsimd, nc.vector),
                        (nc.gpsimd, nc.vector, nc.gpsimd)]
                for H in range(2):
                    sh = slice(CHUNK * H, CHUNK * (H + 1))
                    e_t1, e_u, e_c = engs[H]
                    e_t1.tensor_mul(t1[:, sh], gact[0][:, sh], gact[2][:, sh])
                    e_u.tensor_mul(c4[:, sh], gact[1][:, sh], c4[:, sh])
                    e_c.tensor_add(c4[:, sh], c4[:, sh], t1[:, sh])
                    nc.scalar.activation(tc4[:, sh], c4[:, sh], Tanh)
                    nc.gpsimd.tensor_mul(h4[:, sh], gact[3][:, sh],
                                         tc4[:, sh])

            # ---- history ----
            for t in range(1, T):
                feat4 = sp_.tile([128, 2 * CHUNK], bf, tag="feat4",
                                 bufs=2, name="feat4")
                for H in range(2):
                    sh = slice(CHUNK * H, CHUNK * (H + 1))
                    fp = fpool.tile([128, CHUNK], f32, tag="fp")
                    for L in range(4):
                        j = 4 * H + L
                        nc.tensor.matmul(
                            fp[32 * L:32 * (L + 1), :],
                            omg[:, 32 * (t - 1):32 * t],
                            zsbh[:, CHUNK * j:CHUNK * (j + 1)],
                            start=True, stop=True, tile_position=(0, 32 * L))
                    nc.scalar.activation(feat4[:, sh], fp[:], Tanh,
                                         bias=biases[:, 0:1])
                lstm_step(feat4, 0, split_gact=True)

            # ---- X_15 = Lam @ Zstack (lane-packed, zero-filled rows) ----
            for H in range(2):
                xp = xpool.tile([128, CHUNK], f32, tag="xp")
                for L in range(4):
                    j = 4 * H + L
                    nc.tensor.matmul(
                        xp[:], lamT4[:, 128 * L:128 * (L + 1)],
                        zsb[:, CHUNK * j:CHUNK * (j + 1)],
                        start=(L == 0), stop=(L == 3))
                sl = slice(CHUNK * H, CHUNK * (H + 1))
                nc.vector.tensor_copy(xsb[:, sl], xp[:])
                nc.vector.tensor_copy(xsbh[:, sl], xp[:])

            # ---- prediction ----
            for s in range(1, npred + 1):
                feat4 = sp_.tile([128, 2 * CHUNK], bf, tag="feat4",
                                 bufs=2, name="feat4")
                for H in range(2):
                    sl = slice(CHUNK * H, CHUNK * (H + 1))
                    fp = fpool.tile([128, CHUNK], f32, tag="fp")
                    nc.tensor.matmul(fp[:], cfwTbd[:],
                                     xsbh[:, sl], start=True, stop=True)
                    nc.scalar.activation(feat4[:, sl], fp[:], Tanh,
                                         bias=biases[:, s:s + 1])
                lstm_step(feat4, s, split_gact=True)

                xps = []
                for H in range(2):
                    sl = slice(CHUNK * H, CHUNK * (H + 1))
                    # X update + command/mu rows in one psum group
                    xp = xpool.tile([128, CHUNK], f32, tag="xp")
                    nc.tensor.matmul(xp[:], fTbd[:], xsb[:, sl],
                                     start=True, stop=False)
                    nc.tensor.matmul(xp[:], bcobd[:], h4[:, sl],
                                     start=False, stop=True)
                    # the bf16 shadow feeds the next step's feat matmul —
                    # issue it before either off-chain fp32 copy queues
                    nc.vector.tensor_copy(xsbh[:, sl], xp[:])
                    xps.append(xp)
                for H in range(2):
                    sl = slice(CHUNK * H, CHUNK * (H + 1))
                    nc.vector.tensor_copy(xsb[:, sl], xps[H][:])
                nc.sync.dma_start(stage[s - 1], xsb[8:32, :])

            # ---- epilogue: gather staged rows, products, conv ----
            def gather(dst_ap, r0):
                # DMA APs balance to at most 3 dims -> one DMA per half;
                # alternate the two HWDGE engines so the gathers overlap
                for H in range(2):
                    src_ap = stage[:, r0:r0 + 4, CHUNK * H:CHUNK * (H + 1)]
                    eng = nc.sync if H == 0 else nc.scalar
                    eng.dma_start(
                        dst_ap[:, 2048 * H:2048 * (H + 1)], src_ap)

            gather(cmdA[64:64 + npred, :], 0)    # c3+c4
            gather(cmdA[0:npred, :], 8)          # c3
            gather(cmdA[32:32 + npred, :], 12)   # c4
            gather(musb[0:npred, :], 16)         # mu_x
            gather(musb[npred:2 * npred, :], 20)  # mu_y
            nc.vector.tensor_scalar_add(cmdA[:], cmdA[:],
                                        biases[0:96, cA + 1:cA + 2])
            nc.vector.tensor_mul(qS[:], cmdA[:], cmdA[:])
            for j in range(8):
                sl = slice(CHUNK * j, CHUNK * (j + 1))
                pp = gpool.tile([nconv, CHUNK], f32, tag="gp")
                nc.tensor.matmul(pp[0:nconv, :], convT[:], qS[:, sl],
                                 start=True, stop=True)
                nc.scalar.activation(poutsb[:, sl], pp[0:nconv, :], Ident,
                                     bias=biases[0:nconv, cA:cA + 1])
            nc.scalar.dma_start(out_d[0:2 * npred, :], musb[:])
            nc.sync.dma_start(out_d[2 * npred:, :], poutsb[:])

    return nc


def _get_nc(npred):
    if npred not in _NC_CACHE:
        _NC_CACHE[npred] = _build_nc(npred)
    return _NC_CACHE[npred]


_RUNNER_CACHE = {}


def _get_runner(npred):
    """Cached jitted SPMD executor.  Mirrors the multi-core path of
    concourse.bass2jax.run_bass_via_pjrt, but builds the shard_map-jitted
    callable once so repeat kernel() calls skip retracing (~1.3 s/call)."""
    if npred in _RUNNER_CACHE:
        return _RUNNER_CACHE[npred]

    import jax
    from jax.sharding import Mesh, PartitionSpec
    from jax.experimental.shard_map import shard_map
    from concourse import bass2jax, mybir

    bass2jax.install_neuronx_cc_hook()
    nc = _get_nc(npred)

    partition_name = (nc.partition_id_tensor.name
                      if nc.partition_id_tensor else None)
    in_names, out_names, out_avals, zero_shapes = [], [], [], []
    for alloc in nc.m.functions[0].allocations:
        if not isinstance(alloc, mybir.MemoryLocationSet):
            continue
        name = alloc.memorylocations[0].name
        if alloc.kind == "ExternalInput":
            if name != partition_name:
                in_names.append(name)
        elif alloc.kind == "ExternalOutput":
            shape = tuple(alloc.tensor_shape)
            dtype = mybir.dt.np(alloc.dtype)
            out_names.append(name)
            out_avals.append(jax.core.ShapedArray(shape, dtype))
            zero_shapes.append((shape, dtype))
    n_params = len(in_names)
    n_outs = len(out_names)
    all_in_names = list(in_names) + list(out_names)
    if partition_name is not None:
        all_in_names.append(partition_name)
    donate = tuple(range(n_params, n_params + n_outs))

    def _body(*args):
        operands = list(args)
        if partition_name is not None:
            operands.append(bass2jax.partition_id_tensor())
        outs = bass2jax._bass_exec_p.bind(
            *operands,
            out_avals=tuple(out_avals),
            in_names=tuple(all_in_names),
            out_names=tuple(out_names),
            lowering_input_output_aliases=(),
            sim_require_finite=True,
            sim_require_nnan=True,
            nc=nc,
        )
        return tuple(outs)

    devices = jax.devices()[:NCORES]
    mesh = Mesh(np.asarray(devices), ("core",))
    in_specs = (PartitionSpec("core"),) * (n_params + n_outs)
    out_specs = (PartitionSpec("core"),) * n_outs
    sharded = jax.jit(
        shard_map(_body, mesh=mesh, in_specs=in_specs, out_specs=out_specs,
                  check_rep=False),
        donate_argnums=donate, keep_unused=True)

    def run(in_maps):
        per_core = [[np.asarray(m[name]) for name in in_names]
                    for m in in_maps]
        concat_in = [
            np.concatenate([per_core[c][i] for c in range(NCORES)], axis=0)
            for i in range(n_params)
        ]
        concat_zeros = [np.zeros((NCORES * s[0], *s[1:]), d)
                        for s, d in zero_shapes]
        out_arrs = sharded(*concat_in, *concat_zeros)
        return [
            {name: np.asarray(out_arrs[i]).reshape(
                NCORES, *out_avals[i].shape)[c]
             for i, name in enumerate(out_names)}
            for c in range(NCORES)
        ]

    _RUNNER_CACHE[npred] = run
    return run


# ---------------------------------------------------------------------------
# entry point
# ---------------------------------------------------------------------------

LAST_RESULTS = None


def kernel(
    hist, position_std_x, position_std_y, velocity_std_x, velocity_std_y,
    acceleration_std_x, acceleration_std_y, jerk_std, coef_G, GR,
    cf_W, cf_b, lstm_Wih, lstm_Whh, lstm_bih, lstm_bhh, co_W, co_b, len_pred,
):
    global LAST_RESULTS
    import ml_dtypes

    npred = int(np.asarray(len_pred))
    inp = dict(hist=hist, position_std_x=position_std_x,
               position_std_y=position_std_y, velocity_std_x=velocity_std_x,
               velocity_std_y=velocity_std_y,
               acceleration_std_x=acceleration_std_x,
               acceleration_std_y=acceleration_std_y, jerk_std=jerk_std,
               coef_G=coef_G, GR=GR, cf_W=cf_W, cf_b=cf_b,
               lstm_Wih=lstm_Wih, lstm_Whh=lstm_Whh, lstm_bih=lstm_bih,
               lstm_bhh=lstm_bhh, co_W=co_W, co_b=co_b)

    hist = np.asarray(hist, np.float32)
    Tl, B, _ = hist.shape
    assert (Tl, B) == (T, BATCH), (Tl, B)

    consts, mu_bias = _host_precompute(inp, npred)
    zs = np.concatenate([hist[:, :, 0], hist[:, :, 1]], axis=0)  # (32, B)
    zs = np.ascontiguousarray(zs, np.float32)

    in_maps = []
    for cidx in range(NCORES):
        m = dict(consts)
        zslice = np.ascontiguousarray(zs[:, NPC * cidx:NPC * (cidx + 1)])
        m["zstack"] = zslice
        m["zstackh"] = zslice.astype(ml_dtypes.bfloat16)
        in_maps.append(m)

    run = _get_runner(npred)
    results = run(in_maps)
    LAST_RESULTS = results

    mu_x = np.empty((npred, BATCH), np.float64)
    mu_y = np.empty((npred, BATCH), np.float64)
    p00 = np.empty((npred, BATCH), np.float64)
    p33 = np.empty((npred, BATCH), np.float64)
    p03 = np.empty((npred, BATCH), np.float64)
    for cidx in range(NCORES):
        o = np.asarray(results[cidx]["out"], np.float64)
        sl = slice(NPC * cidx, NPC * (cidx + 1))
        mu_x[:, sl] = o[0:npred]
        mu_y[:, sl] = o[npred:2 * npred]
        p00[:, sl] = o[2 * npred:3 * npred]
        p33[:, sl] = o[3 * npred:4 * npred]
        p03[:, sl] = o[4 * npred:5 * npred]

    mu_x += mu_bias[:, 0:1]
    mu_y += mu_bias[:, 1:2]
    sx = np.sqrt(np.maximum(p00, 0.0))
    sy = np.sqrt(np.maximum(p33, 0.0))
    rho = p03 / np.maximum(sx * sy, 1e-300)
    out = np.stack([mu_x, mu_y, sx, sy, rho], axis=2)
    return out.astype(np.float32)
